# revision 30
# baseline (speedup 1.0000x reference)
"""Trainium2 Bass kernel for nn_AttentionModel: single-head attention with
vocab-sized input/output projections, tensor-parallel across 8 NeuronCores.

Math (reference):
    Q = x @ Wq + bq ; K = x @ Wk + bk ; V = x @ Wv + bv        [S, E]
    scores = Q @ K^T / sqrt(E)                                  [S, S]
    out = softmax(scores) @ V @ Wo + bo                         [S, VOCAB]

Sharding: vocab dim (50257, padded to 8*6400) split across 8 cores.
  Phase A : per-core partial K^T = Wk_c^T @ x_c^T   -> AllReduce (chunked
            per 512-seq block so each chunk's AR overlaps later compute)
  Phase Bq: per-core partial Q = x_c @ Wq_c(scaled) -> ReduceScatter
            (fires right after Bq; hidden under the whole Bv pass)
  Phase Bv: per-core partial V = x_c @ Wv_c         -> AllReduce (chunked
            per 512-seq rows, overlapping Bv itself)
  Phase C : attention on this core's 256-query slice: scores^T = K @ Q_s^T,
            exp (no max subtraction: scores ~N(0,1)), denominators via
            ones-matmul, recip computed locally (bf16 hi+lo pair), then
            unnormalized ctx^T = V^T @ exp^T.  ctx^T + recip -> AllGather
            in two 128-query column chunks.
  Phase D : out_c = (ctx @ Wo_c) * recip, one chunk per AG half.
All matmuls run bf16 inputs with fp32 PSUM accumulation.  1/sqrt(E) is folded
into Wq host-side; Q/K/V biases ride a ones-row in the padded vocab dim.
DMAs are batched into large multi-tile transfers (HWDGE descriptor-generation
costs a fixed ~625ns per DMA instruction, so fewer/bigger is strictly better;
all contiguous runs are kept >=512B to stay off the slow descriptor path).
Output is written bf16 and upcast host-side.
"""

import sys

if "/opt/trn_rl_repo" not in sys.path:
    sys.path.insert(0, "/opt/trn_rl_repo")

import numpy as np
import ml_dtypes

import concourse.bass as bass
import concourse.tile as tile
from concourse import bacc, mybir
from concourse import bass_utils
from concourse.masks import make_identity

BF16 = mybir.dt.bfloat16
F32 = mybir.dt.float32
NP_BF16 = ml_dtypes.bfloat16


class Cfg:
    def __init__(self, S=2048, E=768, VS=6400, n_cores=8, vocab=50257):
        assert S % 512 == 0 and E % 128 == 0 and VS % 128 == 0
        self.S = S  # full sequence
        self.E = E  # embed dim
        self.VS = VS  # padded vocab rows per core
        self.n_cores = n_cores
        self.vocab = vocab
        self.ST = S // 128  # seq tiles
        self.ET = E // 128  # embed tiles
        self.KT = VS // 128  # contraction (vocab) tiles per core
        self.QS = S // n_cores  # queries per core
        assert self.QS % 128 == 0
        self.QT = self.QS // 128
        self.SC = S // 512  # 512-wide seq chunks (phase A)
        self.KC = 10  # vocab k-tiles per x-load chunk
        # chunk list with ramped-up sizes so the first matmuls of a pass
        # aren't queued behind multi-MB DMAs
        ch, k0 = [], 0
        for sz in [2, 4, 8]:
            ch.append((k0, k0 + sz))
            k0 += sz
        while k0 < self.KT:
            sz = min(self.KC, self.KT - k0)
            ch.append((k0, k0 + sz))
            k0 += sz
        self.kch = ch
        # phase D vocab chunks (over this core's VS output columns)
        self.nch = [(i * 512, min(512, VS - i * 512)) for i in range((VS + 511) // 512)]


FULL = Cfg()


def build_nc(cfg: Cfg, reps: int = 1, emulate_cc: bool = False):
    S, E, VS = cfg.S, cfg.E, cfg.VS
    ST, ET, KT, QS, QT, SC, KC = cfg.ST, cfg.ET, cfg.KT, cfg.QS, cfg.QT, cfg.SC, cfg.KC
    RG = [list(range(cfg.n_cores))]
    NKC = KT // KC  # x-load chunks over the contraction
    BLK = E + 128  # AG block rows: ctx^T rows plus a 128-row recip pad

    nc = bacc.Bacc(None, target_bir_lowering=False, num_devices=cfg.n_cores)

    xT = nc.dram_tensor("xT", [VS, S], BF16, kind="ExternalInput")
    wq = nc.dram_tensor("wq", [VS, E], BF16, kind="ExternalInput")
    wk = nc.dram_tensor("wk", [VS, E], BF16, kind="ExternalInput")
    wv = nc.dram_tensor("wv", [VS, E], BF16, kind="ExternalInput")
    wo = nc.dram_tensor("wo", [E, VS], BF16, kind="ExternalInput")
    out = nc.dram_tensor("out", [S, VS], BF16, kind="ExternalOutput")

    xT_t = xT.ap().rearrange("(kt p) s -> p kt s", p=128)
    wq_t = wq.ap().rearrange("(kt p) e -> p kt e", p=128)
    wk_t = wk.ap().rearrange("(kt p) e -> p kt e", p=128)
    wv_t = wv.ap().rearrange("(kt p) e -> p kt e", p=128)
    wo_t = wo.ap().rearrange("(et p) v -> p et v", p=128)
    out_t = out.ap().rearrange("(st p) v -> p st v", p=128)

    # internal DRAM for collectives.  Collectives carry a large fixed cost on
    # this stack (~25us each, unoverlapped and size-independent at these
    # sizes), so there are exactly three: AllReduce(K^T|V concatenated flat,
    # after Bv), ReduceScatter(Q, after Bq), AllGather(ctx).
    ES = E * S
    kv_in = nc.dram_tensor("kv_in", [2 * ES], BF16)
    kv_out = nc.dram_tensor("kv_out", [2 * ES], BF16, addr_space="Shared")
    q_in = nc.dram_tensor("q_in", [S, E], BF16)
    q_out = nc.dram_tensor("q_out", [QS, E], BF16)
    ctx_in = nc.dram_tensor("ctx_in", [QT * BLK, 128], BF16)
    ctx_out = nc.dram_tensor(
        "ctx_out", [cfg.n_cores * QT * BLK, 128], BF16, addr_space="Shared"
    )
    kt_in_t = kv_in.ap()[0:ES].rearrange("(et p s) -> p et s", et=ET, p=128)
    kt_out_t = kv_out.ap()[0:ES].rearrange("(et p s) -> p et s", et=ET, p=128)
    v_in_t = kv_in.ap()[ES : 2 * ES].rearrange("(t p e) -> p t e", t=ST, p=128)
    v_out_t = kv_out.ap()[ES : 2 * ES].rearrange("(t p e) -> p t e", t=ST, p=128)
    ctx_in_t = ctx_in.ap().rearrange("(c p) q -> p c q", p=128)
    ctx_out_t = ctx_out.ap().rearrange("(c p) q -> p c q", p=128)

    def do_cc(kind, in_t, out_t):
        if not emulate_cc:
            op = (
                mybir.AluOpType.bypass
                if kind == "AllGather"
                else mybir.AluOpType.add
            )
            nc.gpsimd.collective_compute(
                kind,
                op,
                replica_groups=RG,
                ins=[in_t.ap().opt()],
                outs=[out_t.ap().opt()],
            )
            return
        # single-core emulation with plain DMA (preserves deps for TimelineSim)
        ish, osh = in_t.shape, out_t.shape
        if kind == "AllReduce":
            if len(ish) == 1:
                nc.sync.dma_start(out=out_t.ap(), in_=in_t.ap())
            else:
                nc.sync.dma_start(out=out_t[:, :], in_=in_t[:, :])
        elif kind == "ReduceScatter":
            nc.sync.dma_start(out=out_t[:, :], in_=in_t[0 : osh[0], :])
        elif kind == "AllGather":
            for c in range(cfg.n_cores):
                nc.sync.dma_start(
                    out=out_t[c * ish[0] : (c + 1) * ish[0], :], in_=in_t[:, :]
                )

    with tile.TileContext(nc) as tc:
        const = tc.alloc_tile_pool(name="const", bufs=1)
        id128 = const.tile([128, 128], BF16)
        make_identity(nc, id128)
        ones = const.tile([128, 1], BF16)
        nc.vector.memset(ones, 1.0)

        for rep in range(reps):
            # ------------- Phase A: partial K^T = Wk_c^T @ x_c^T -------------
            wkp = tc.alloc_tile_pool(name="wkp", bufs=1)
            wk_sb = wkp.tile([128, KT, E], BF16)
            wqp = tc.alloc_tile_pool(name="wqp", bufs=1, side="right")
            wq_sb = wqp.tile([128, KT, E], BF16)
            xa = tc.alloc_tile_pool(name="xa", bufs=3)
            evA = tc.alloc_tile_pool(name="evA", bufs=2)
            psA = tc.alloc_tile_pool(name="psA", bufs=8, space="PSUM")

            for sc in range(SC):
                ps_list = [
                    psA.tile([128, 512], F32, name=f"psa_{sc}_{i}", tag="psa")
                    for i in range(ET)
                ]
                for gi, (k0, k1) in enumerate(cfg.kch):
                    # JIT-load wk per chunk on the first pass
                    if sc == 0:
                        nc.sync.dma_start(
                            out=wk_sb[:, k0:k1, :], in_=wk_t[:, k0:k1, :]
                        )
                    xt = xa.tile([128, KC, 512], BF16)
                    nc.sync.dma_start(
                        out=xt[:, 0 : k1 - k0, :],
                        in_=xT_t[:, k0:k1, sc * 512 : (sc + 1) * 512],
                    )
                    # trickle the wq prefetch through later passes, after the
                    # x load so it doesn't starve the x-tile feed
                    if sc >= 2 and gi in (1, 3, 5):
                        gq = (sc - 2) * 3 + (gi - 1) // 2
                        if gq < NKC:
                            nc.sync.dma_start(
                                out=wq_sb[:, gq * KC : (gq + 1) * KC, :],
                                in_=wq_t[:, gq * KC : (gq + 1) * KC, :],
                            )
                    for kk in range(k1 - k0):
                        k = k0 + kk
                        for em in range(ET):
                            nc.tensor.matmul(
                                ps_list[em],
                                lhsT=wk_sb[:, k, em * 128 : (em + 1) * 128],
                                rhs=xt[:, kk, :],
                                start=(k == 0),
                                stop=(k == KT - 1),
                            )
                stg = evA.tile([128, ET, 512], BF16)
                for em in range(ET):
                    nc.vector.tensor_copy(stg[:, em, :], ps_list[em])
                nc.sync.dma_start(
                    out=kt_in_t[:, :, sc * 512 : (sc + 1) * 512], in_=stg
                )
            psA.release()
            evA.release()
            xa.release()
            wkp.release()

            # ------------- Phase Bq: partial Q = x_c @ Wq_c -------------
            wvp = tc.alloc_tile_pool(name="wvp", bufs=1)
            wv_sb = wvp.tile([128, KT, E], BF16)

            def phase_b(w_sb, dst_t, prefetch_w, prefetch_t, extra=None):
                xb = tc.alloc_tile_pool(name="xb", bufs=3)
                evB = tc.alloc_tile_pool(name="evB", bufs=2)
                psB = tc.alloc_tile_pool(name="psB", bufs=4, space="PSUM")
                for mg in range(ST // 2):
                    if extra is not None:
                        extra(mg)
                    ps2 = [
                        psB.tile([128, E], F32, name=f"psb_{mg}_{i}", tag="psb")
                        for i in range(2)
                    ]
                    for gi, (k0, k1) in enumerate(cfg.kch):
                        xt2 = xb.tile([128, KC, 256], BF16)
                        nc.sync.dma_start(
                            out=xt2[:, 0 : k1 - k0, :],
                            in_=xT_t[:, k0:k1, mg * 256 : (mg + 1) * 256],
                        )
                        # spread the next-phase weight prefetch: two chunks
                        # per mg from mg=2, emitted after the x load
                        if prefetch_w is not None and mg >= 2 and gi in (2, 5):
                            gw = (mg - 2) * 2 + (1 if gi == 5 else 0)
                            if gw < NKC:
                                nc.sync.dma_start(
                                    out=prefetch_w[:, gw * KC : (gw + 1) * KC, :],
                                    in_=prefetch_t[:, gw * KC : (gw + 1) * KC, :],
                                )
                        for kk in range(k1 - k0):
                            k = k0 + kk
                            for m2 in range(2):
                                for c0 in range(0, E, 512):
                                    c1 = min(c0 + 512, E)
                                    nc.tensor.matmul(
                                        ps2[m2][:, c0:c1],
                                        lhsT=xt2[:, kk, m2 * 128 : (m2 + 1) * 128],
                                        rhs=w_sb[:, k, c0:c1],
                                        start=(k == 0),
                                        stop=(k == KT - 1),
                                    )
                    stg = evB.tile([128, 2, E], BF16)
                    for m2 in range(2):
                        nc.vector.tensor_copy(stg[:, m2, :], ps2[m2])
                    nc.sync.dma_start(
                        out=dst_t[:, mg * 2 : mg * 2 + 2, :], in_=stg
                    )
                psB.release()
                evB.release()
                xb.release()

            phase_b(wq_sb, q_in.ap().rearrange("(t p) e -> p t e", p=128), wv_sb, wv_t)
            wqp.release()
            do_cc("ReduceScatter", q_in, q_out)

            # ------------- Phase Bv: partial V = x_c @ Wv_c -------------
            # prefetch wo + the AR'd K^T + the RS'd Q slice during Bv
            wop = tc.alloc_tile_pool(name="wop", bufs=1, side="right")
            wo_sb = wop.tile([128, ET, VS], BF16)
            kvp = tc.alloc_tile_pool(name="kvp", bufs=1, side="right")
            kt_sb = kvp.tile([128, ET, S], BF16)
            qstg = kvp.tile([128, QT, E], BF16)

            def bv_extra(mg):
                if mg == 0:
                    nc.sync.dma_start(
                        out=qstg,
                        in_=q_out.ap().rearrange("(t p) e -> p t e", p=128),
                    )
                elif mg <= ET:
                    et = mg - 1
                    nc.sync.dma_start(out=wo_sb[:, et, :], in_=wo_t[:, et, :])

            phase_b(wv_sb, v_in_t, None, None, extra=bv_extra)
            for mg in range(ST // 2, 1 + ET):  # any prefetch slices missed
                bv_extra(mg)
            do_cc("AllReduce", kv_in, kv_out)
            wvp.release()

            # ------------- Phase C: attention on this core's query slice ----
            ap_ = tc.alloc_tile_pool(name="attnp", bufs=1)
            cstage = tc.alloc_tile_pool(name="cstage", bufs=4)
            psC = tc.alloc_tile_pool(name="psC", bufs=2, space="PSUM")
            # K^T and V reloads, chunked so they stream in while the qT
            # transposes / scores matmuls run
            for c in range(SC):
                nc.sync.dma_start(
                    out=kt_sb[:, :, c * 512 : (c + 1) * 512],
                    in_=kt_out_t[:, :, c * 512 : (c + 1) * 512],
                )
            v_sb = ap_.tile([128, ST, E], BF16)
            for c in range(SC):
                nc.sync.dma_start(
                    out=v_sb[:, c * 4 : (c + 1) * 4, :],
                    in_=v_out_t[:, c * 4 : (c + 1) * 4, :],
                )
            # Q_s -> transpose to [E, QS] bf16
            qT_sb = ap_.tile([128, ET, QS], BF16)
            for qt in range(QT):
                for et in range(ET):
                    ps_t = psC.tile([128, 128], BF16, tag="pst")
                    nc.tensor.transpose(
                        ps_t, qstg[:, qt, et * 128 : (et + 1) * 128], id128
                    )
                    nc.vector.tensor_copy(
                        qT_sb[:, et, qt * 128 : (qt + 1) * 128], ps_t
                    )
            # scores^T tiles + exp
            expT_sb = ap_.tile([128, ST, QS], BF16)
            for mk in range(ST):
                ps_s = psC.tile([128, QS], F32, tag="psf", bufs=3)
                for et in range(ET):
                    nc.tensor.matmul(
                        ps_s,
                        lhsT=kt_sb[:, et, mk * 128 : (mk + 1) * 128],
                        rhs=qT_sb[:, et, :],
                        start=(et == 0),
                        stop=(et == ET - 1),
                    )
                nc.scalar.activation(
                    out=expT_sb[:, mk, :],
                    in_=ps_s,
                    func=mybir.ActivationFunctionType.Exp,
                )
            # denominators, directly transposed: denom^T[q] = sum_k exp^T[k,q]
            # (exp^T block stationary, ones as the moving operand), then local
            # reciprocal carried as a bf16 hi+lo pair
            recT = [cstage.tile([128, 2], BF16, name=f"recT{h}") for h in range(QT)]
            for h in range(QT):
                ps_dh = psC.tile([128, 1], F32, tag="pst")
                for mk in range(ST):
                    nc.tensor.matmul(
                        ps_dh,
                        lhsT=expT_sb[:, mk, h * 128 : (h + 1) * 128],
                        rhs=ones,
                        start=(mk == 0),
                        stop=(mk == ST - 1),
                    )
                rec = cstage.tile([128, 1], F32)
                nc.vector.reciprocal(rec, ps_dh)
                rhi_f = cstage.tile([128, 1], F32)
                nc.vector.tensor_copy(recT[h][:, 0:1], rec)
                nc.vector.tensor_copy(rhi_f, recT[h][:, 0:1])
                rlo_f = cstage.tile([128, 1], F32)
                nc.vector.tensor_sub(rlo_f, rec, rhi_f)
                nc.vector.tensor_copy(recT[h][:, 1:2], rlo_f)
            # unnormalized ctx^T = V^T @ exp^T
            cstg = cstage.tile([128, ET, QS], BF16)
            for et in range(ET):
                ps_c = psC.tile([128, QS], F32, tag="psf", bufs=3)
                for mk in range(ST):
                    nc.tensor.matmul(
                        ps_c,
                        lhsT=v_sb[:, mk, et * 128 : (et + 1) * 128],
                        rhs=expT_sb[:, mk, :],
                        start=(mk == 0),
                        stop=(mk == ST - 1),
                    )
                nc.vector.tensor_copy(cstg[:, et, :], ps_c)
            NB = BLK // 128  # 128-row blocks per h-slab
            for h in range(QT):
                nc.sync.dma_start(
                    out=ctx_in_t[:, h * NB : h * NB + ET, :],
                    in_=cstg[:, :, h * 128 : (h + 1) * 128],
                )
                nc.sync.dma_start(
                    out=ctx_in[h * BLK + E : h * BLK + E + 128, 0:2], in_=recT[h]
                )
            do_cc("AllGather", ctx_in, ctx_out)
            psC.release()
            cstage.release()
            ap_.release()
            kvp.release()

            # ------------- Phase D: out_c = ctx @ Wo_c * recip -------------
            CB = QT * NB  # 128-row blocks per core block in ctx_out
            ctxp = tc.alloc_tile_pool(name="ctxp", bufs=1)
            osb = tc.alloc_tile_pool(name="osb", bufs=2)
            psD = tc.alloc_tile_pool(name="psD", bufs=8, space="PSUM")
            ctx_all = ctxp.tile([128, cfg.n_cores * CB, 128], BF16)
            rsum = ctxp.tile([128, ST, 1], F32)
            for h in range(QT):
                for blk in range(cfg.n_cores):
                    c0 = blk * CB + h * NB
                    nc.sync.dma_start(
                        out=ctx_all[:, c0 : c0 + NB, :],
                        in_=ctx_out_t[:, c0 : c0 + NB, :],
                    )
            for h in range(QT):
                for blk in range(cfg.n_cores):
                    ms = blk * QT + h
                    c0 = blk * CB + h * NB
                    nc.vector.tensor_add(
                        rsum[:, ms, :],
                        ctx_all[:, c0 + ET, 0:1],
                        ctx_all[:, c0 + ET, 1:2],
                    )
                    ost = osb.tile([128, VS], BF16)
                    half = len(cfg.nch) // 2
                    for ni, (n0, nsz) in enumerate(cfg.nch):
                        ps_o = psD.tile([128, 512], F32)
                        for et in range(ET):
                            nc.tensor.matmul(
                                ps_o[:, :nsz],
                                lhsT=ctx_all[:, c0 + et, :],
                                rhs=wo_sb[:, et, n0 : n0 + nsz],
                                start=(et == 0),
                                stop=(et == ET - 1),
                            )
                        nc.vector.tensor_scalar_mul(
                            ost[:, n0 : n0 + nsz], ps_o[:, :nsz], rsum[:, ms, :]
                        )
                        if ni == half - 1:
                            nc.sync.dma_start(
                                out=out_t[:, ms, 0 : cfg.nch[half][0]],
                                in_=ost[:, 0 : cfg.nch[half][0]],
                            )
                    nc.sync.dma_start(
                        out=out_t[:, ms, cfg.nch[half][0] :],
                        in_=ost[:, cfg.nch[half][0] :],
                    )
            psD.release()
            osb.release()
            ctxp.release()
            wop.release()

        const.release()

    nc.compile()
    return nc


def _shard_bounds(cfg: Cfg):
    base = cfg.vocab // cfg.n_cores
    rem = cfg.vocab % cfg.n_cores
    sizes = [base + (1 if c < rem else 0) for c in range(cfg.n_cores)]
    starts = [sum(sizes[:c]) for c in range(cfg.n_cores)]
    return starts, sizes


def prepare_inputs(cfg: Cfg, x, Wq, bq, Wk, bk, Wv, bv, Wo):
    """Host-side shard/pad/cast. Returns in_maps for run_bass_kernel_spmd."""
    S, E, VS, N = cfg.S, cfg.E, cfg.VS, cfg.n_cores
    inv = np.float32(1.0 / np.sqrt(E))
    xT = np.ascontiguousarray(x.reshape(S, -1).T.astype(np.float32)).astype(NP_BF16)
    Wq_s = (Wq.astype(np.float32) * inv).astype(NP_BF16)
    Wk_s = Wk.astype(np.float32).astype(NP_BF16)
    Wv_s = Wv.astype(np.float32).astype(NP_BF16)
    Wo_s = Wo.astype(np.float32).astype(NP_BF16)
    bq_s = (bq.astype(np.float32) * inv / N).astype(np.float32)
    bk_s = (bk.astype(np.float32) / N).astype(np.float32)
    bv_s = (bv.astype(np.float32) / N).astype(np.float32)

    starts, sizes = _shard_bounds(cfg)
    in_maps = []
    for c in range(N):
        s0, rv = starts[c], sizes[c]
        assert rv <= VS - 1, "need a free padded row for the bias/ones row"
        xs = np.zeros((VS, S), dtype=NP_BF16)
        xs[:rv] = xT[s0 : s0 + rv]
        xs[VS - 1] = NP_BF16(1.0)
        wqc = np.zeros((VS, E), dtype=NP_BF16)
        wqc[:rv] = Wq_s[s0 : s0 + rv]
        wqc[VS - 1] = bq_s.astype(NP_BF16)
        wkc = np.zeros((VS, E), dtype=NP_BF16)
        wkc[:rv] = Wk_s[s0 : s0 + rv]
        wkc[VS - 1] = bk_s.astype(NP_BF16)
        wvc = np.zeros((VS, E), dtype=NP_BF16)
        wvc[:rv] = Wv_s[s0 : s0 + rv]
        wvc[VS - 1] = bv_s.astype(NP_BF16)
        woc = np.zeros((E, VS), dtype=NP_BF16)
        woc[:, :rv] = Wo_s[:, s0 : s0 + rv]
        in_maps.append({"xT": xs, "wq": wqc, "wk": wkc, "wv": wvc, "wo": woc})
    return in_maps


def assemble_output(cfg: Cfg, results, bo):
    starts, sizes = _shard_bounds(cfg)
    parts = [
        results[c]["out"][:, : sizes[c]].astype(np.float32)
        for c in range(cfg.n_cores)
    ]
    full = np.concatenate(parts, axis=1)
    full = full + bo.astype(np.float32)[None, :]
    return full[None].astype(np.float32)


_NC_CACHE = {}


def _get_nc(cfg: Cfg):
    key = (cfg.S, cfg.E, cfg.VS, cfg.n_cores)
    if key not in _NC_CACHE:
        _NC_CACHE[key] = build_nc(cfg)
    return _NC_CACHE[key]


def kernel(x, Wq, bq, Wk, bk, Wv, bv, Wo, bo):
    cfg = FULL
    x = np.asarray(x)
    in_maps = prepare_inputs(
        cfg,
        x,
        np.asarray(Wq),
        np.asarray(bq),
        np.asarray(Wk),
        np.asarray(bk),
        np.asarray(Wv),
        np.asarray(bv),
        np.asarray(Wo),
    )
    nc = _get_nc(cfg)
    res = bass_utils.run_bass_kernel_spmd(
        nc, in_maps, core_ids=list(range(cfg.n_cores))
    )
    return assemble_output(cfg, res.results, np.asarray(bo))
